# revision 42
# baseline (speedup 1.0000x reference)
"""Trainium2 Bass kernel for single-step causal GQA attention with KV cache.

Problem (hardcoded shapes):
  x[4,16,4096] @ Wq/Wk/Wv -> RoPE -> append to KV cache (start_pos=2048) ->
  GQA attention over T=2064 keys -> @ Wo -> out[4,16,4096], fp32 in/out.

Sharding (8 cores, tensor-parallel over heads):
  core c gets q-heads 4c..4c+3 (Wq cols c*512:(c+1)*512), kv-head c
  (Wk/Wv cols c*128:(c+1)*128, cache [:, :, c, :]), and Wo rows
  c*512:(c+1)*512 (row-parallel). The 8 partial outputs are summed on
  the host (which also undoes the transposed output layout).

Precision: the kernel is HBM-bound (the cost model's DMA pipe is
360 GB/s/core), so precision is spent where the DMA bytes are: fp16 for
x/Wq/Wk/Wv/K-cache (fp16's 10 mantissa bits beat bf16's 8 for free),
and float8_e3m4 for the two most byte-heavy, error-tolerant operands:
Wo and the V cache (the PE allows mixed-dtype operands, so fp8 V/Wo
multiply directly against fp16 expT/attnT). Both fp8 tensors are scaled
by a per-core power of two on the host so their absmax sits at ~12
(e3m4 max 15.5); the scales fold out for free: Wv ships pre-scaled by
the V-cache scale so new tokens match the cache, the softmax
denominator's ones-column stays 1.0 so attn comes out lam_v-scaled, and
the host divides each core's partial by lam_v*lam_wo after the gather.
Measured end-to-end rel err ~1.7e-2 (gate 2e-2).

Matmul orientation: the cost model charges a matmul by its MOVING rows
(output free size) only, so every projection keeps the 64-token axis
moving: Q/K/V are computed as W.T @ x -> [dims, tokens] (27ns each
instead of 213ns), which also lands q/k directly in the transposed
layout the score matmuls want, and Wo as Wo.T @ attnT -> [out-dims,
tokens] with the transposed result fixed up on the host. RoPE runs in
"split" layout: the host permutes each head's dims so rotation pairs
sit 64 partitions apart, making the rotation six plain partition-sliced
DVE ops straight out of PSUM (no pair-swap views, no PE transposes).
The K cache and Wk/Wq columns ship with the same dim permutation so
score dot products stay consistent; V/Wo are untouched by it.

Scheduling notes (what actually set the runtime):
  - 19 input DMAs + 5 output DMAs, all HWDGE. Every HWDGE DMA ticks one
    of 8 global round-robin semaphore lanes by 16 and lane sems saturate
    around 63, so >24 HWDGE DMAs forces the tile framework to insert a
    mid-program all-engine barrier; 24 keeps every lane at <=3 ticks.
  - Every DMA moves >=512B-contiguous runs (smaller runs are charged 2x
    bandwidth). V ships as one merged fp8 tensor [128,B,NT,HDP] with its
    softmax-denominator ones column pre-padded per chunk ([..,HD]=1); K
    ships per batch (4 DMAs) so batch b's scores start as soon as its
    keys land; Wo is packed block-contiguous (4*128 = 512B runs at fp8).
  - Load order = consumption-chain length: wkv early, wq in shrinking
    groups (the last wq byte gates the whole attention phase), K0/K1,
    then V, K2/K3, then the Wo blocks tapering (3x1024, 512, 256,
    2x128) so the final arrival's dependent chain (4 matmuls of 64
    rows + one small copy + one flush) is the kernel tail.
  - The attention batch loop is software-pipelined: batch b+1's score
    matmuls are emitted before batch b's AV matmuls. PSUM ring
    discipline sets the batch cadence: per batch the 'sc' tag carries
    exactly three allocations (scores u=0/u=1 + the attnT transpose)
    and the new-token score tile lives in the 'ou' tag, so batch b+1's
    first allocation only waits on batch b's first exp, not its whole
    chain. All PSUM lives in three tags (3+2+3 banks).
  - The flipped Wo phase accumulates 1024 output cols (8 x 128-col
    groups) per full-bank PSUM tile; copies go out per 2 groups
    alternating ACT/DVE, the last unit splitting 2+1+1 so the final
    wo arrival's chain is short. Flush pieces are emitted after all
    copies and alternate SP/ACT so their ~650ns HWDGE gens overlap.
"""

import math

import numpy as np

import concourse.bass as bass
import concourse.mybir as mybir
import concourse.tile as tile
from concourse import bacc
from concourse.bass_utils import run_bass_kernel_spmd
from concourse.masks import make_identity

F32 = mybir.dt.float32
F16 = mybir.dt.float16
F8 = mybir.dt.float8e3

B, S, DIM = 4, 16, 4096
NH, NKV, HD = 32, 8, 128
START = 2048
BS = B * S              # 64 tokens
NCORES = 8
QH = NH // NCORES       # 4 q heads per core
QW = QH * HD            # 512 = per-core Wq width
KC = DIM // 128         # 32 contraction chunks
NT = START // 128       # 16 full cache chunks per batch
HDP = HD + 1            # V row padded with ones column at HD
HH = HD // 2            # 64 = split-RoPE half
TW = QH * S             # 64 = scoresT free width (h-major, then s)
EXPW = NT * TW + TW     # 1088 = expT tile width (16 cache chunks + new chunk)
SCALE = 1.0 / math.sqrt(HD)
F8MAX_TARGET = 12.0     # scale fp8 tensors so absmax lands in (6, 12]
WQ_GROUPS = [12, 12, 8]      # wq DMA group sizes in KC chunks
# wo column blocks (col, width): big early, small late so the final
# arrival's dependent chain is as short as possible
WO_BLOCKS = [(0, 1024), (1024, 1024), (2048, 1024), (3072, 512),
             (3584, 256), (3840, 128), (3968, 128)]
# flush pieces (col-group range, issue engine) over the transposed
# output [128, 32 groups, 64]; the final group ships straight from PSUM
# (fp32, separate tensor) so the tail chain has no copy in it
WO_PIECES = [(0, 8, "sync"), (8, 16, "scalar"), (16, 24, "sync"),
             (24, 31, "scalar")]


def build_bass() -> bass.Bass:
    nc = bacc.Bacc()

    xT = nc.dram_tensor("xT", [128, KC, BS], F16, kind="ExternalInput")
    wq = nc.dram_tensor("wq", [128, KC, QW], F16, kind="ExternalInput")
    wk = nc.dram_tensor("wk", [128, KC, HD], F16, kind="ExternalInput")
    # wv ships fp8, scaled by lam_v*mu (pow2); v_new is descaled by 1/mu
    # via the data-driven mu tile (per-core constants can't live in the
    # shared SPMD program)
    wv = nc.dram_tensor("wv", [128, KC, HD], F8, kind="ExternalInput")
    mu = nc.dram_tensor("mu", [128, 1], F32, kind="ExternalInput")
    # wo is packed block-contiguous by the host: block i's [128, 4, w]
    # slab occupies cols [4*col, 4*(col+w))
    wo = nc.dram_tensor("wo", [128, 4 * DIM], F8, kind="ExternalInput")
    kT = nc.dram_tensor("kT", [B, 128, START], F16, kind="ExternalInput")
    v8 = nc.dram_tensor("v8", [128, B, NT, HDP], F8, kind="ExternalInput")
    # split-RoPE tables: [64, 2, 256 (q: h,t)] cc|ss then [64, 2, 64 (k: t)]
    ropes = nc.dram_tensor("ropes", [HH, 2, 256 + BS], F16,
                           kind="ExternalInput")
    # output is transposed: [128 out-dims, 32 col-groups, 64 tokens];
    # the host untransposes (free) after the gather
    out = nc.dram_tensor("out", [128, DIM // 128, BS], F16,
                         kind="ExternalOutput")

    with tile.TileContext(nc) as tc:
        with (
            tc.tile_pool(name="const", bufs=1) as const,
            tc.tile_pool(name="wqp", bufs=4) as wqp,
            tc.tile_pool(name="wkvp", bufs=1) as wkvp,
            tc.tile_pool(name="wop", bufs=3) as wop,
            tc.tile_pool(name="kvp", bufs=4) as kvp,
            tc.tile_pool(name="acts", bufs=1) as acts,
            tc.tile_pool(name="expp", bufs=3) as expp,
            tc.tile_pool(name="small", bufs=4) as small,
            tc.tile_pool(name="ps_sc", bufs=3, space="PSUM") as ps_sc,
            tc.tile_pool(name="ps_ou", bufs=3, space="PSUM") as ps_ou,
            tc.tile_pool(name="ps_wo", bufs=2, space="PSUM") as ps_wo,
        ):
            ident = const.tile([128, 128], F32, tag="ident")
            make_identity(nc, ident[:])
            # touch Exp once so the ACT LUT loads during phase 1, not on
            # the batch-0 softmax critical path
            warm = const.tile([1, 4], F32, tag="warm")
            nc.scalar.activation(
                warm[:], ident[:1, :4], mybir.ActivationFunctionType.Exp
            )

            # ---- the 19 input DMAs, SP queue, in consumption order ----
            xT_sb = const.tile([128, KC, BS], F16, tag="xT")
            nc.sync.dma_start(xT_sb[:], xT.ap())

            ropes_sb = const.tile([HH, 2, 256 + BS], F16, tag="ropes")
            nc.sync.dma_start(ropes_sb[:], ropes.ap())
            mu_sb = const.tile([128, 1], F32, tag="mu")
            nc.sync.dma_start(mu_sb[:], mu.ap())
            ccq = ropes_sb[:, 0, :256]
            ssq = ropes_sb[:, 1, :256]
            cck = ropes_sb[:, 0, 256:]
            ssk = ropes_sb[:, 1, 256:]

            # wk first (short chain to kTn), then wq (its last byte gates
            # the whole attention phase), then wv (v_new needed only by
            # av(0)), then K/V interleaved so the last batch's keys and
            # values land as early as the stream allows
            wk_sb = wkvp.tile([128, KC, HD], F16, tag="wk")
            nc.sync.dma_start(wk_sb[:], wk.ap())

            wq_tiles = []
            c0 = 0
            for g, gw in enumerate(WQ_GROUPS):
                wq_sb = wqp.tile([128, gw, QW], F16, tag=f"wq{gw}",
                                 name="wq_sb")
                nc.sync.dma_start(wq_sb[:], wq.ap()[:, c0 : c0 + gw, :])
                wq_tiles.append((c0, wq_sb))
                c0 += gw

            wv_sb = wkvp.tile([128, KC, HD], F8, tag="wv")
            nc.sync.dma_start(wv_sb[:], wv.ap())

            # K/V interleaved by consumption: kT0, kT1, V(b0/b1), kT2,
            # kT3 split in two (its exps are the serial tail), V(b2/b3)
            kT_tiles = {}
            v8_sb = kvp.tile([128, B, NT, HDP], F8, tag="v8", name="v8_sb")
            for b in range(2):
                kT_sb = kvp.tile([128, START], F16, tag="kT", name="kT_sb")
                nc.sync.dma_start(kT_sb[:], kT.ap()[b])
                kT_tiles[b] = kT_sb
            nc.sync.dma_start(v8_sb[:, :2], v8.ap()[:, :2])
            kT_sb = kvp.tile([128, START], F16, tag="kT", name="kT_sb")
            nc.sync.dma_start(kT_sb[:], kT.ap()[2])
            kT_tiles[2] = kT_sb
            kT_sb = kvp.tile([128, START], F16, tag="kT", name="kT_sb")
            nc.sync.dma_start(kT_sb[:, : START // 2],
                              kT.ap()[B - 1][:, : START // 2])
            nc.sync.dma_start(kT_sb[:, START // 2 :],
                              kT.ap()[B - 1][:, START // 2 :])
            kT_tiles[B - 1] = kT_sb
            nc.sync.dma_start(v8_sb[:, 2:], v8.ap()[:, 2:])

            # Wo column blocks, arriving last (their consumers are last)
            wo_sb = []
            for col, w in WO_BLOCKS:
                wo_t = wop.tile([128, 4, w], F8, tag=f"wo{col}", bufs=1,
                                name="wo_t")
                nc.sync.dma_start(wo_t[:], wo.ap()[:, 4 * col : 4 * (col + w)])
                wo_sb.append(wo_t)

            attnT = acts.tile([128, QH, BS], F16, tag="attnT")

            def rope_split(dst, raw, cc, ss, w, tmp_tag):
                # dst[0:64]  = raw[0:64]*cc - raw[64:128]*ss   (DVE)
                # dst[64:..] = raw[0:64]*ss + raw[64:128]*cc   (Pool)
                # GPSIMD can't read PSUM, so one ACT copy stages `raw` to
                # SBUF; then the two halves run on DVE and Pool in
                # parallel (three elementwise ops each instead of six
                # serial on DVE)
                rt = acts.tile([HH, w], F32, tag=tmp_tag + "rt")
                rb = acts.tile([HH, w], F32, tag=tmp_tag + "rb")
                nc.scalar.copy(rt[:], raw[:HH])
                nc.scalar.copy(rb[:], raw[HH:])
                t0 = acts.tile([HH, w], F32, tag=tmp_tag + "0")
                t1 = acts.tile([HH, w], F32, tag=tmp_tag + "1")
                t2 = acts.tile([HH, w], F32, tag=tmp_tag + "2")
                t3 = acts.tile([HH, w], F32, tag=tmp_tag + "3")
                nc.vector.tensor_mul(t0[:], rt[:], cc)
                nc.gpsimd.tensor_mul(t2[:], rt[:], ss)
                nc.vector.tensor_mul(t1[:], rb[:], ss)
                nc.gpsimd.tensor_mul(t3[:], rb[:], cc)
                nc.vector.tensor_sub(dst[:HH], t0[:], t1[:])
                nc.gpsimd.tensor_add(dst[HH:], t2[:], t3[:])

            # ---- projections, flipped: W.T @ x -> [dims, tokens], in
            # data-arrival order (K, Q, V) so the in-order PE queue never
            # stalls a ready matmul behind a waiting one. k lands in kT
            # layout (no transpose), RoPE applied in split layout
            # straight out of PSUM ----
            kn_ps = ps_ou.tile([128, BS], F32, tag="ou", name="kn_ps")
            for c in range(KC):
                nc.tensor.matmul(
                    kn_ps[:],
                    lhsT=wk_sb[:, c, :],
                    rhs=xT_sb[:, c, :],
                    start=(c == 0),
                    stop=(c == KC - 1),
                )
            kTn_sb = acts.tile([128, BS], F16, tag="kTn")
            rope_split(kTn_sb, kn_ps, cck, ssk, BS, "rk")

            # ---- Q projection, chasing the wq DMA groups ----
            qraw_ps = ps_wo.tile([128, QH, BS], F32, tag="wops",
                                 name="qraw_ps")
            for c0, wq_sb in wq_tiles:
                for j in range(wq_sb.shape[1]):
                    c = c0 + j
                    for h in range(QH):
                        nc.tensor.matmul(
                            qraw_ps[:, h, :],
                            lhsT=wq_sb[:, j, 128 * h : 128 * (h + 1)],
                            rhs=xT_sb[:, c, :],
                            start=(c == 0 and h == 0),
                            stop=(c == KC - 1 and h == QH - 1),
                        )
            qT_sb = acts.tile([128, QH, BS], F16, tag="qT")
            rope_split(
                qT_sb[:].rearrange("p h t -> p (h t)"),
                qraw_ps[:].rearrange("p h t -> p (h t)"),
                ccq, ssq, QH * BS, "rq",
            )

            # ---- attention, software-pipelined: batch b+1's score matmuls
            # are emitted before batch b's AV so the PE never waits through
            # the exp round-trip. The V projection + v_new regroup are
            # emitted AFTER scores(0) (they're only needed by av(0), and
            # the in-order PE queue must not park scores(0) behind the
            # wv/vT waits) ----
            expTs = {}

            def emit_scores(b):
                kT_sb = kT_tiles[b]
                qT_b = qT_sb[:, :, 16 * b : 16 * (b + 1)]  # [128,4,16]
                expT = expp.tile([128, EXPW], F16, tag="expT", name="expT")
                for u in range(2):  # one exp per 8 chunks
                    sc = ps_sc.tile([128, 8, TW], F32, tag="sc", name="sc")
                    for j in range(8):
                        t = 8 * u + j
                        nc.tensor.matmul(
                            sc[:, j, :],
                            lhsT=kT_sb[:, 128 * t : 128 * (t + 1)],
                            rhs=qT_b,
                            start=True,
                            stop=True,
                        )
                    nc.scalar.activation(
                        expT[:, 8 * TW * u : 8 * TW * (u + 1)],
                        sc[:],
                        mybir.ActivationFunctionType.Exp,
                        scale=SCALE,
                    )
                # new-token scores live in the 'ou' tag so the 'sc' ring
                # stays at 3 allocations per batch (no inter-batch stall)
                scn = ps_ou.tile([S, TW], F32, tag="ou", name="scn")
                nc.tensor.matmul(
                    scn[:],
                    lhsT=kTn_sb[:, 16 * b : 16 * (b + 1)],
                    rhs=qT_b,
                    start=True,
                    stop=True,
                )
                nc.scalar.activation(
                    expT[:S, NT * TW :],
                    scn[:],
                    mybir.ActivationFunctionType.Exp,
                    scale=SCALE,
                )
                expTs[b] = expT

            def emit_av(b):
                expT = expTs.pop(b)
                # unnormalized out [tok(h,s), hd | exp-sum col at HD];
                # V is fp8 (mixed-dtype matmul vs fp16 expT), its ones
                # column is 1.0 so the denominator is unscaled and attn
                # comes out lam_v-scaled (host descales after Wo)
                ou = ps_ou.tile([BS, HDP], F32, tag="ou", name="ou")
                for t in range(NT):
                    nc.tensor.matmul(
                        ou[:, : HD + 1],
                        lhsT=expT[:, TW * t : TW * (t + 1)],
                        rhs=v8_sb[:, b, t, :],
                        start=(t == 0),
                        stop=False,
                    )
                nc.tensor.matmul(
                    ou[:, : HD + 1],
                    lhsT=expT[:S, NT * TW :],
                    rhs=v_new[:, b, : HD + 1],
                    start=False,
                    stop=True,
                )

                # rcp on DVE (ACT Reciprocal is banned for accuracy); the
                # normalizing copy runs on ACT with a per-partition scale
                rcp = small.tile([TW, 1], F32, tag="rcp")
                nc.vector.reciprocal(rcp[:], ou[:, HD : HD + 1])
                attn = small.tile([TW, HD], F32, tag="attn")
                nc.scalar.activation(
                    attn[:], ou[:, :HD],
                    mybir.ActivationFunctionType.Copy, scale=rcp[:],
                )

                # one strided copy into attnT per batch
                aps = ps_sc.tile([128, QH, S], F32, tag="sc", name="aps")
                nc.tensor.transpose(
                    aps[:].rearrange("p h s -> p (h s)"), attn[:],
                    ident[:TW, :TW],
                )
                nc.vector.tensor_copy(attnT[:, :, 16 * b : 16 * (b + 1)], aps[:])

            emit_scores(0)
            emit_scores(1)

            # ---- V projection + v_new regroup (needed first by av(0)) ----
            vn_ps = ps_ou.tile([128, BS], F32, tag="ou", name="vn_ps")
            for c in range(KC):
                nc.tensor.matmul(
                    vn_ps[:],
                    lhsT=wv_sb[:, c, :],
                    rhs=xT_sb[:, c, :],
                    start=(c == 0),
                    stop=(c == KC - 1),
                )
            # psb lives in the 'wops' ring: putting it in 'sc' couples the
            # V regroup to the score/exp ring and serializes the batches
            vT_sb = acts.tile([128, BS], F32, tag="vT")
            nc.vector.tensor_scalar_mul(vT_sb[:], vn_ps[:], mu_sb[:])
            v_new = acts.tile([S, B, HDP], F16, tag="v_new")
            nc.vector.memset(v_new[:, :, HD : HD + 1], 1.0)
            for b in range(B):
                psb = ps_wo.tile([S, 128], F32, tag="wops", name="psb")
                nc.tensor.transpose(
                    psb[:], vT_sb[:, 16 * b : 16 * (b + 1)], ident[:]
                )
                nc.vector.tensor_copy(v_new[:, b, :HD], psb[:])

            emit_av(0)
            emit_scores(2)
            emit_av(1)
            emit_scores(3)
            emit_av(2)
            emit_av(3)

            # ---- output projection, flipped: oT[128 od, 64 tok] per
            # 128-col group = woT.T @ attnT; units of 8 groups (1024 Wo
            # cols) share one full-bank PSUM accumulation ----
            oT_sb = acts.tile([128, DIM // 128, BS], F16, tag="oT")
            blk_of = {}
            for i, (col, w) in enumerate(WO_BLOCKS):
                for g in range(w // 128):
                    blk_of[col + 128 * g] = (i, col)
            ncopy = 0
            for u in range(4):
                wo_ps = ps_wo.tile([128, 8, BS], F32, tag="wops",
                                   name="wo_ps")
                for g in range(8):
                    gcol = 1024 * u + 128 * g
                    i, bcol = blk_of[gcol]
                    off = gcol - bcol
                    for k in range(4):
                        nc.tensor.matmul(
                            wo_ps[:, g, :],
                            lhsT=wo_sb[i][:, k, off : off + 128],
                            rhs=attnT[:, k, :],
                            start=(g == 0 and k == 0),
                            stop=(g == 7 and k == 3),
                        )
                # one big copy per unit (per-op sem/SEQ overhead dominates
                # small copies); the last unit splits 7+1 so the final wo
                # arrival's chain is 4 matmuls + a tiny copy + one flush
                chunks = ((0, 8),) if u < 3 else ((0, 7), (7, 8))
                for g0, g1 in chunks:
                    eng = (nc.scalar.copy if ncopy % 2 == 0
                           else nc.vector.tensor_copy)
                    eng(
                        oT_sb[:, 8 * u + g0 : 8 * u + g1, :],
                        wo_ps[:, g0:g1, :],
                    )
                    ncopy += 1
            # flush the pieces AFTER all copies in program order: a piece
            # DMA's semaphore wait parked on the SP/ACT queue must never
            # starve a later copy on the same queue
            for g0, g1, eng_name in WO_PIECES:
                dma_eng = getattr(nc, eng_name)
                dma_eng.dma_start(
                    out.ap()[:, g0:g1, :], oT_sb[:, g0:g1, :]
                )
            nc.sync.dma_start(out.ap()[:, 31:, :], oT_sb[:, 31:, :])

    nc.compile()
    return nc


# split-RoPE dim permutation: position i <- old dim 2i, position 64+i <-
# old dim 2i+1 (per 128-dim head)
PERM = np.concatenate([np.arange(0, HD, 2), np.arange(1, HD, 2)])


def _rope_tables(freqs_cos, freqs_sin):
    # [HH, 2, 256+BS] fp16: q tables [HH, 256 (h,t)] then k tables [HH, BS]
    cos = np.asarray(freqs_cos, np.float32).T  # [HH, S]
    sin = np.asarray(freqs_sin, np.float32).T
    ccq = np.tile(cos, (1, QH * B)).reshape(HH, QH * B, S)
    ssq = np.tile(sin, (1, QH * B)).reshape(HH, QH * B, S)
    # q free layout is (h, b, s): tile over (h*b) then s fastest
    ccq = ccq.reshape(HH, 256)
    ssq = ssq.reshape(HH, 256)
    cck = np.tile(cos, (1, B)).reshape(HH, BS)
    ssk = np.tile(sin, (1, B)).reshape(HH, BS)
    t = np.stack(
        [np.concatenate([ccq, cck], 1), np.concatenate([ssq, ssk], 1)], 1
    )
    return t.astype(np.float16)


def _f16(a):
    return np.ascontiguousarray(a).astype(np.float16)


def _pmaj(w):
    # [KC*128, N] -> [128, KC, N]: per-partition-contiguous SBUF order
    kc, n = w.shape[0] // 128, w.shape[1]
    return w.reshape(kc, 128, n).transpose(1, 0, 2)


_BASS_CACHE = {}


def make_in_maps(x, freqs_cos, freqs_sin, cache_k, cache_v, Wq, Wk, Wv, Wo):
    import ml_dtypes

    x = np.ascontiguousarray(np.asarray(x, np.float32))
    cache_k = np.asarray(cache_k, np.float32)
    cache_v = np.asarray(cache_v, np.float32)
    Wq = np.asarray(Wq, np.float32)
    Wk = np.asarray(Wk, np.float32)
    Wv = np.asarray(Wv, np.float32)
    Wo = np.asarray(Wo, np.float32)

    xT = _f16(x.reshape(BS, KC, 128).transpose(2, 1, 0))
    ropes = np.ascontiguousarray(_rope_tables(freqs_cos, freqs_sin))

    in_maps, descales = [], []
    for c in range(NCORES):
        # K cache with the split-RoPE dim permutation on its hd axis
        kTc = _f16(cache_k[:, :START, c, :][:, :, PERM].transpose(0, 2, 1))
        # V cache, fp8, with the softmax-denominator ones column padded
        # per chunk and a power-of-2 scale lam_v; layout [128, B, NT, HDP]
        vc = cache_v[:, :START, c, :]
        lam_v = 2.0 ** np.floor(np.log2(F8MAX_TARGET / np.abs(vc).max()))
        v4 = np.zeros((B, NT, 128, HDP), np.float32)
        v4[..., :HD] = vc.reshape(B, NT, 128, HD) * lam_v
        v4[..., HD] = 1.0
        v8 = np.ascontiguousarray(v4.transpose(2, 0, 1, 3)).astype(
            ml_dtypes.float8_e3m4
        )
        # Wo packed block-contiguous, fp8 with power-of-2 scale lam_wo
        wo_p = np.concatenate(
            [
                Wo[QW * c : QW * (c + 1), col : col + w]
                .reshape(4, 128, w).transpose(1, 0, 2)
                .reshape(128, 4 * w)
                for col, w in WO_BLOCKS
            ],
            axis=1,
        )
        lam_wo = 2.0 ** np.floor(np.log2(F8MAX_TARGET / np.abs(wo_p).max()))
        wo8 = (wo_p * lam_wo).astype(ml_dtypes.float8_e3m4)
        # Wq/Wk columns carry the same split-RoPE dim permutation as the
        # K cache. Wv ships fp8, scaled by lam_v (cache match) times its
        # own pow2 mu for fp8 range; the kernel descales v_new by 1/mu
        # via the mu tile.
        wq_c = Wq[:, QW * c : QW * (c + 1)].reshape(DIM, QH, HD)[
            :, :, PERM
        ].reshape(DIM, QW)
        wv_s = _pmaj(Wv[:, HD * c : HD * (c + 1)]) * lam_v
        mu_c = 2.0 ** np.floor(np.log2(F8MAX_TARGET / np.abs(wv_s).max()))
        wv8 = (wv_s * mu_c).astype(ml_dtypes.float8_e3m4)
        in_maps.append(
            {
                "xT": xT,
                "wq": _f16(_pmaj(wq_c)),
                "wk": _f16(_pmaj(Wk[:, HD * c : HD * (c + 1)][:, PERM])),
                "wv": np.ascontiguousarray(wv8),
                "mu": np.full((128, 1), 1.0 / mu_c, np.float32),
                "wo": wo8,
                "kT": kTc,
                "v8": v8,
                "ropes": ropes,
            }
        )
        descales.append(1.0 / (lam_v * lam_wo))
    return in_maps, descales


def unshard_out(raw):
    # [128, 32, 64] transposed partial -> [BS, DIM]
    return (
        np.asarray(raw, np.float32).transpose(2, 1, 0).reshape(BS, DIM)
    )


def kernel(x, freqs_cos, freqs_sin, cache_k, cache_v, Wq, Wk, Wv, Wo, start_pos):
    assert int(start_pos) == START
    in_maps, descales = make_in_maps(
        x, freqs_cos, freqs_sin, cache_k, cache_v, Wq, Wk, Wv, Wo
    )
    if "nc" not in _BASS_CACHE:
        _BASS_CACHE["nc"] = build_bass()
    res = run_bass_kernel_spmd(
        _BASS_CACHE["nc"], in_maps, core_ids=list(range(NCORES))
    )
    total = np.zeros((BS, DIM), np.float32)
    for r, d in zip(res.results, descales):
        total += unshard_out(r["out"]) * d
    return total.reshape(B, S, DIM)
